# revision 12
# baseline (speedup 1.0000x reference)
"""Trainium2 Bass kernel for a bidirectional multi-head LSTM block.

Model (B=128, T=197, D=768, 12 heads x 64 hid):
    y   = x @ pre_w.T + pre_b
    hf  = LSTM_fwd(y)   (block-diagonal per-head gate weights)
    hr  = LSTM_rev(y)
    out = concat(hf, hr) @ proj_w.T + proj_b

Sharding over 8 NeuronCores: (4 head-groups of 3 heads) x (2 batch
halves of 64). Each core runs BOTH directions for its heads/batch —
two independent recurrence chains that pipeline against each other.

Launch 1 (per core), all fp16 matmul operands / fp32 PSUM:
  - phase A: y = x @ preW.T streamed into persistent SBUF tiles
    (yA: heads 0,1 on partitions 0:128; yB: head 2 on 0:64).
  - w_ih gate contributions computed in 2-step bursts into a 3-slot
    PSUM ring per direction (LDWEIGHTS amortized, start=True);
    the per-step w_hh matmuls accumulate on top (start=False,stop=True),
    with the gate bias riding row 64 of the h ring.
  - gates: ONE Sigmoid ACT per dir/step over the whole [128, 2F] block.
    g-gate weight rows are pre-scaled x2 so sigma(2x) is computed, and
    tanh(x) = 2*sigma(2x) - 1 is recovered with a tensor_scalar op.
    Pointwise then uses plain tensor_tensor ops (2x DVE mode in fp16):
        u = sig_f * c;  tg = 2*sig_g2 - 1;  v = sig_i * tg
        c' = u + v;  tc = tanh(c');  h = sig_o * tc
    Partition-base choreography (2-input DVE ops need equal in bases):
        dir0: chunk0 = [i|f] (i@0:64, f@64:128), chunk1 = [g|o]; c @ rows 64:128
        dir1: chunk0 = [f|i],                    chunk1 = [o|g]; c @ rows 0:64
  - h history kept in a 16-slot SBUF ring; DMA'd to DRAM every 8 steps.
    dir-1 h is stored by STEP index; the host reverses its time axis.

Launch 2 (per core): batch-slice output projection
out_slice.T = proj_w_aug @ lstm_out_slice_aug (fp16, K padded to 1664,
ones/bias row folds in proj_b).
"""

import os
from contextlib import ExitStack

import numpy as np

import concourse.bass as bass
import concourse.tile as tile
from concourse import bacc, mybir
from concourse.bass_utils import run_bass_kernel_spmd
from concourse.kernels.tile_matmul import matmul_tile_kernel

B, T, D = 128, 197, 768
HEADS, HPH = 12, 64
H = HEADS * HPH  # 768
NCORES = 8
NG = 4                 # head groups
HPG = HEADS // NG      # 3 heads per group
GD = HPG * HPH         # 192 pre-proj cols per group
BC = B // 2            # 64 batch per core
F = HPG * BC           # 192: free dim of one gate chunk (3 heads x 64)
F32 = mybir.dt.float32
F16 = mybir.dt.float16
ADD = mybir.AluOpType.add
MULT = mybir.AluOpType.mult
SIGM = mybir.ActivationFunctionType.Sigmoid
TANH = mybir.ActivationFunctionType.Tanh

NTILE = 512            # phase-A chunk (cols of (t, b))
HRING = 16             # h-ring slots; DMA every HRING//2 steps
HDMA = HRING // 2

LAST_RESULTS = []      # stash of BassKernelResults for test harnesses


# --------------------------------------------------------------------------
# Program 1: pre-projection + bidirectional LSTM recurrence
# --------------------------------------------------------------------------
def build_lstm_program(t_steps=T):
    TBc = t_steps * BC
    NKT = D // 128         # 6 k-tiles of the pre-projection
    n_ntiles = (TBc + NTILE - 1) // NTILE
    steps_per_ntile = NTILE // BC   # 4

    nc = bacc.Bacc("TRN2", target_bir_lowering=False, debug=False)

    xT = nc.dram_tensor("xT", [D, TBc], F16, kind="ExternalInput")
    preWT = nc.dram_tensor("preWT", [D, GD], F16, kind="ExternalInput")
    # lhsT blocks per (dir d, head j, chunk k) at col 128*((d*HPG+j)*2+k)
    whh = nc.dram_tensor("whh", [HPH + 1, 2 * HPG * 2 * 128], F16,
                         kind="ExternalInput")
    # rows 0:64 = w_ih lhsT blocks; rows 64:128 = duplicate (head-1 rhs base)
    wih = nc.dram_tensor("wih", [128, 2 * HPG * 2 * 128], F16,
                         kind="ExternalInput")
    # [r, dir, step, head j, b]   (dir 1 indexed by step; host flips time)
    hsT = nc.dram_tensor("hsT", [HPH, 2, t_steps, HPG, BC], F16,
                         kind="ExternalOutput")

    xT_k = xT.rearrange("(k p) n -> p k n", p=128)            # [128, 6, TBc]

    with tile.TileContext(nc) as tc, ExitStack() as ctx:
        # ---------------- pools ----------------
        const = ctx.enter_context(tc.tile_pool(name="const", bufs=1))
        xk_pool = ctx.enter_context(tc.tile_pool(name="xk", bufs=3))
        paA = ctx.enter_context(tc.tile_pool(name="paA", bufs=1, space="PSUM"))
        paB = ctx.enter_context(tc.tile_pool(name="paB", bufs=1, space="PSUM"))
        rec_ps = ctx.enter_context(tc.tile_pool(name="rec_ps", bufs=3, space="PSUM"))
        gpool = ctx.enter_context(tc.tile_pool(name="gpool", bufs=2))
        state = ctx.enter_context(tc.tile_pool(name="state", bufs=1))

        # ---------------- constants / state ----------------
        pw_sb = const.tile([128, NKT * GD], F16, tag="pw", name="pw_sb")
        pw_sb3 = pw_sb.rearrange("p (k m) -> p k m", k=NKT)
        nc.sync.dma_start(pw_sb3[:], preWT.rearrange("(k p) m -> p k m", p=128))

        whh_sb = const.tile([HPH + 1, 2 * HPG * 2 * 128], F16, tag="whh",
                            name="whh_sb")
        nc.sync.dma_start(whh_sb[:], whh[:])
        wih_sb = const.tile([128, 2 * HPG * 2 * 128], F16, tag="wih",
                            name="wih_sb")
        nc.sync.dma_start(wih_sb[:], wih[:])

        # persistent y (phase-A output): one base-0 tile per head,
        # cols = t*BC + b.
        yH = [
            const.tile([64, TBc], F16, tag=f"yH{j}", name=f"yH{j}")
            for j in range(HPG)
        ]

        # per-dir c-state, base 0
        ST = []
        for d in range(2):
            st_d = state.tile([64, F], F16, tag=f"ST{d}", name=f"ST{d}")
            nc.vector.memset(st_d[:], 0.0)
            ST.append(st_d)

        # h rings: rows 0:64 h (slot cols), row 64 = ones (bias row)
        h_ring = []
        for d in range(2):
            hr = state.tile([HPH + 1, HRING * F], F16, tag=f"hr{d}",
                            name=f"h_ring{d}")
            nc.vector.memset(hr[0:HPH, :], 0.0)
            nc.vector.memset(hr[HPH : HPH + 1, :], 1.0)
            h_ring.append(hr)

        # ---------------- phase-A emission ----------------
        def emit_ntile(ni):
            n0 = ni * NTILE
            nsz = min(NTILE, TBc - n0)
            xk = xk_pool.tile([128, NKT * NTILE], F16, tag="xk", name="xk")
            xk3 = xk.rearrange("p (k n) -> p k n", k=NKT)
            nc.sync.dma_start(xk3[:, :, 0:nsz], xT_k[:, :, n0 : n0 + nsz])
            psA = paA.tile([128, NTILE], F32, tag="psA", name="psA")
            psB = paB.tile([64, NTILE], F32, tag="psB", name="psB")
            for k in range(NKT):
                nc.tensor.matmul(
                    psA[:, 0:nsz], pw_sb3[:, k, 0:128], xk3[:, k, 0:nsz],
                    start=(k == 0), stop=(k == NKT - 1),
                )
            for k in range(NKT):
                nc.tensor.matmul(
                    psB[:, 0:nsz], pw_sb3[:, k, 128:GD], xk3[:, k, 0:nsz],
                    start=(k == 0), stop=(k == NKT - 1),
                )
            nc.scalar.copy(yH[0][:, n0 : n0 + nsz], psA[0:64, 0:nsz])
            nc.vector.tensor_copy(yH[1][:, n0 : n0 + nsz], psA[64:128, 0:nsz])
            nc.scalar.copy(yH[2][:, n0 : n0 + nsz], psB[:, 0:nsz])

        front = 0            # next front tile to emit
        back = n_ntiles - 1  # next back tile to emit

        def ensure_tiles(t):
            nonlocal front, back
            # bursts reach t+2 (pair t+1, t+2); cover to t+4 with margin
            want_front = min(n_ntiles - 1, (t + 4) // steps_per_ntile + 1)
            lo = ((t_steps - 1 - (t + 4)) * BC) // NTILE
            want_back = max(0, lo - 1)
            while front <= back and (front <= want_front or back >= want_back):
                if front <= want_front:
                    emit_ntile(front)
                    front += 1
                if front > back:
                    break
                if back >= want_back:
                    emit_ntile(back)
                    back -= 1

        # ---------------- w_ih bursts ----------------
        ps_tiles = [{}, {}]      # per dir: step -> psum ring tile

        def burst(d, t0, nsteps):
            """w_ih gate contributions for steps [t0, t0+nsteps) of dir d."""
            steps = [s for s in range(t0, t0 + nsteps) if s < t_steps]
            for s in steps:
                ps_tiles[d][s] = rec_ps.tile(
                    [128, 2 * F], F32, tag=f"ps{d}", name=f"ps{d}"
                )
            for j in range(HPG):
                for k in range(2):
                    blk = 128 * ((d * HPG + j) * 2 + k)
                    lhsT = wih_sb[0:64, blk : blk + 128]
                    for s in steps:
                        t_eff = s if d == 0 else t_steps - 1 - s
                        rhs = yH[j][:, t_eff * BC : (t_eff + 1) * BC]
                        # one accumulation group per psum bank: the first MM
                        # into a ring tile opens it (zeroing the bank), the
                        # last w_hh MM in rec_step closes it
                        nc.tensor.matmul(
                            ps_tiles[d][s][:, k * F + j * BC : k * F + (j + 1) * BC],
                            lhsT, rhs, start=(j == 0 and k == 0), stop=False,
                        )

        # ---------------- one recurrence step of one direction ----------
        def rec_step(d, t):
            slot = t % HRING
            pslot = (t - 1) % HRING
            ps = ps_tiles[d].pop(t)
            hr = h_ring[d]

            # w_hh (+bias row) accumulate into the burst-initialized regions
            for j in range(HPG):
                for k in range(2):
                    blk = 128 * ((d * HPG + j) * 2 + k)
                    nc.tensor.matmul(
                        ps[:, k * F + j * BC : k * F + (j + 1) * BC],
                        whh_sb[:, blk : blk + 128],
                        hr[:, pslot * F + j * BC : pslot * F + (j + 1) * BC],
                        start=False, stop=(j == HPG - 1 and k == 1),
                    )

            gt = gpool.tile([128, 2 * F], F16, tag=f"gt{d}", name=f"gt{d}")
            nc.scalar.activation(gt[:], ps[:], SIGM)

            # chunk0 = [f|i] (f@0:64, i@64:128), chunk1 = [o|g] (o@0, g@64)
            # all DVE ops partition-base aligned (base-shifted DVE ops run
            # ~2-3.5x slower); the one unavoidable 64->0 crossover (v) runs
            # on the otherwise-idle GpSimd engine.
            sig_f = gt[0:64, 0:F]
            sig_i = gt[64:128, 0:F]
            sig_o = gt[0:64, F : 2 * F]
            sig_g2 = gt[64:128, F : 2 * F]
            st_d = ST[d]

            tmp = gpool.tile([128, 4 * F], F16, tag=f"tmp{d}", name=f"tmp{d}")
            tg = tmp[64:128, 0:F]
            v = tmp[0:64, F : 2 * F]
            u = tmp[0:64, 2 * F : 3 * F]
            tc_t = tmp[0:64, 3 * F : 4 * F]

            # tg = 2*sig_g2 - 1    (tanh of the g preactivation)   @64
            nc.vector.tensor_scalar(tg, sig_g2, 2.0, -1.0, MULT, ADD)
            # v = sig_i * tg   (the 64->0 crossover, on GpSimd)
            nc.gpsimd.tensor_tensor(v, sig_i, tg, MULT)
            # u = sig_f * c    @0
            nc.vector.tensor_tensor(u, sig_f, st_d[:], MULT)
            # c' = u + v       @0
            nc.vector.tensor_tensor(st_d[:], u, v, ADD)
            # tc = tanh(c')    @0
            nc.scalar.activation(tc_t, st_d[:], TANH)
            # h = sig_o * tc   @0
            nc.vector.tensor_tensor(
                hr[0:HPH, slot * F : (slot + 1) * F], sig_o, tc_t, MULT
            )

            # batched h writeback (ring cols are already (step, head, b))
            if t % HDMA == HDMA - 1 or t == t_steps - 1:
                s0 = t - (t % HDMA)
                cnt = t - s0 + 1
                r0 = s0 % HRING
                src = hr[0:HPH, r0 * F : (r0 + cnt) * F].rearrange(
                    "p (s hb) -> p s hb", s=cnt
                )
                nc.sync.dma_start(
                    hsT[:, d, s0 : s0 + cnt].rearrange("p s h b -> p s (h b)"),
                    src,
                )

        # ---------------- time loop ----------------
        ensure_tiles(0)
        burst(0, 0, 2)
        burst(1, 0, 2)
        for t in range(t_steps):
            ensure_tiles(t)
            rec_step(0, t)
            rec_step(1, t)
            if t % 2 == 1 and t + 1 < t_steps:
                burst(0, t + 1, 2)
                burst(1, t + 1, 2)
        while front <= back:
            emit_ntile(front)
            front += 1

    nc.compile()
    return nc


# --------------------------------------------------------------------------
# Program 2: output projection for a batch slice
# --------------------------------------------------------------------------
KPAD = 13 * 128            # 1664 (>= 2H+1 bias row)


def build_proj_program():
    Bc = B // NCORES       # 16
    TBc2 = T * Bc          # 3152
    nc = bacc.Bacc("TRN2", target_bir_lowering=False, debug=False)
    lstmT = nc.dram_tensor("lstmT", [KPAD, TBc2], F16, kind="ExternalInput")
    projWT = nc.dram_tensor("projWT", [KPAD, D], F16, kind="ExternalInput")
    outT = nc.dram_tensor("outT", [D, TBc2], F32, kind="ExternalOutput")
    with tile.TileContext(nc) as tc:
        matmul_tile_kernel(tc, projWT[:], lstmT[:], outT[:])
    nc.compile()
    return nc


# --------------------------------------------------------------------------
# Host-side weight prep
# --------------------------------------------------------------------------
def _head_rows(h, order):
    """Gate rows of head h in the [4H, *] weights for the given chunk order
    (pytorch row order is i,f,g,o)."""
    gate_idx = {"i": 0, "f": 1, "g": 2, "o": 3}
    rows = [
        np.arange(gate_idx[g] * H + h * HPH, gate_idx[g] * H + (h + 1) * HPH)
        for g in order
    ]
    return np.concatenate(rows)       # 256 rows


def _prep_weights(pre_w, pre_b, dirs, heads):
    """preWT [768,GD]; whh [65, 2*6*128]; wih [128, 2*6*128] for one core."""
    preWT = np.concatenate(
        [pre_w[h * HPH : (h + 1) * HPH, :] for h in heads], axis=0
    ).T.copy()
    whh = np.zeros((HPH + 1, 2 * HPG * 2 * 128), np.float32)
    wih = np.zeros((128, 2 * HPG * 2 * 128), np.float32)
    # row scale: x2 on g rows (sigma(2x) trick); chunk orders per dir
    orders = [("f", "i", "o", "g"), ("f", "i", "o", "g")]
    for d, (w_ih, w_hh, b_ih, b_hh) in enumerate(dirs):
        order = orders[d]
        scale = np.concatenate(
            [np.full(64, 2.0 if g == "g" else 1.0) for g in order]
        ).astype(np.float32)
        for j, h in enumerate(heads):
            rows = _head_rows(h, order)
            cols = np.arange(h * HPH, (h + 1) * HPH)
            Wih = w_ih[np.ix_(rows, cols)] * scale[:, None]          # [256,64]
            Whh = w_hh[np.ix_(rows, cols)] * scale[:, None]
            bias = (Wih @ pre_b[cols]) + (b_ih[rows] + b_hh[rows]) * scale
            for k in range(2):
                blk = 128 * ((d * HPG + j) * 2 + k)
                sl = slice(k * 128, (k + 1) * 128)
                whh[0:HPH, blk : blk + 128] = Whh[sl, :].T
                whh[HPH, blk : blk + 128] = bias[sl]
                wih[0:64, blk : blk + 128] = Wih[sl, :].T
                wih[64:128, blk : blk + 128] = Wih[sl, :].T
    return (preWT.astype(np.float16), whh.astype(np.float16),
            wih.astype(np.float16))


# --------------------------------------------------------------------------
# Main entry
# --------------------------------------------------------------------------
def kernel(
    x,
    pre_w,
    pre_b,
    w_ih_f,
    w_hh_f,
    b_ih_f,
    b_hh_f,
    w_ih_r,
    w_hh_r,
    b_ih_r,
    b_hh_r,
    proj_w,
    proj_b,
):
    trace = bool(os.environ.get("KERNEL_TRACE"))
    LAST_RESULTS.clear()
    core_ids = list(range(NCORES))
    dirs = [
        (w_ih_f, w_hh_f, b_ih_f, b_hh_f),
        (w_ih_r, w_hh_r, b_ih_r, b_hh_r),
    ]

    # ---- launch 1 inputs: core c = (group g, batch-half bh)
    xT_bh = [
        np.ascontiguousarray(
            x[bh * BC : (bh + 1) * BC].transpose(2, 1, 0).reshape(D, T * BC)
        ).astype(np.float16)
        for bh in range(2)
    ]
    in_maps1 = []
    for c in core_ids:
        g, bh = divmod(c, 2)
        heads = [g * HPG + j for j in range(HPG)]
        preWT, whh, wih = _prep_weights(pre_w, pre_b, dirs, heads)
        in_maps1.append(
            {"xT": xT_bh[bh], "preWT": preWT, "whh": whh, "wih": wih}
        )

    nc1 = build_lstm_program()
    res1 = run_bass_kernel_spmd(nc1, in_maps1, core_ids, trace=trace)
    LAST_RESULTS.append(res1)

    # ---- assemble lstm_out rows [1536, B, T]
    lstm_rows = np.empty((2 * H, B, T), np.float16)
    for c in core_ids:
        g, bh = divmod(c, 2)
        hs = res1.results[c]["hsT"]          # [64, 2, T, 3, 64]
        hs = hs.copy()
        hs[:, 1] = hs[:, 1, ::-1]            # dir-1 stored by step; flip time
        for d in range(2):
            arr = hs[:, d].transpose(2, 0, 3, 1)      # [3, 64, BC, T]
            r0 = d * H + g * HPG * HPH
            lstm_rows[r0 : r0 + GD, bh * BC : (bh + 1) * BC] = arr.reshape(
                GD, BC, T
            )

    # ---- launch 2
    Bc = B // NCORES
    TBc2 = T * Bc
    projWT = np.zeros((KPAD, D), np.float16)
    projWT[: 2 * H] = proj_w.T.astype(np.float16)
    projWT[2 * H] = proj_b
    in_maps2 = []
    for c in core_ids:
        kxn = np.zeros((KPAD, TBc2), np.float16)
        kxn[: 2 * H] = lstm_rows[:, c * Bc : (c + 1) * Bc, :].reshape(2 * H, TBc2)
        kxn[2 * H] = 1.0
        in_maps2.append({"lstmT": kxn, "projWT": projWT})

    nc2 = build_proj_program()
    res2 = run_bass_kernel_spmd(nc2, in_maps2, core_ids, trace=trace)
    LAST_RESULTS.append(res2)

    out = np.empty((B, T, D), np.float32)
    for c in core_ids:
        outT = res2.results[c]["outT"]       # [768, 3152]
        out[c * Bc : (c + 1) * Bc] = outT.reshape(D, Bc, T).transpose(1, 2, 0)
    return out
